# revision 13
# baseline (speedup 1.0000x reference)
"""DetectionLoss Trainium2 Bass kernel (v3: sparse CE, single gather).

Data-parallel over batch: 2 images per core x 8 cores; host sums 18 partial
sums per core (npos is a global normalizer, so per-core normalization is
impossible anyway - the sharding hint's "per-shard sums + counts").

The cross-entropy term only touches POSITIVE cells (<=128 per scale per
core), so no dense pass over the cls logits is needed.  Host repacks
obj/reg/cls into per-cell records (pure relayout - all arithmetic happens on
device).  Because floor(x*40) == floor(x*80)>>1 exactly in f32 (identical
mantissas), the s1/s2 cells are determined by the s0 cell, so the three
scales' records are concatenated per s0-cell into one [12800, 108] table and
ONE indirect gather fetches obj+reg+cls[30] for all scales.  logsumexp /
smooth-L1 / CE then run on the 128 gathered rows.  Only the objectness BCE
is dense: obj logits arrive packed [128, 132] (pad = -1e4 contributes
softplus 0).

Other latency cuts vs the dense version:
- one packed [128, 48] input DMA (boxes, labels-as-f32, per-scale consts,
  class iota) instead of four small ones: DMA *issue* costs ~0.7us each.
- identity / upper-tri / matmul-selector constants generated on device via
  memset + affine_select instead of DMAed.
- winner masks and min-label reductions batched over scales as [128,3,128];
  the per-scale key/label row-matrices come from one PE transpose of
  [keyf0|keyf1|keyf2|labf] plus 4 selector matmuls.
- scalar queue is forced (false deps) to run Exp ops before all Ln ops:
  the engine has one activation-table slot, each load costs 1.28us.
- floor via (x - 0.5 + 1.5*2^23) - 1.5*2^23: the sum sits in [2^23, 2^24)
  where f32 ulp is 1; plain 2^23 leaves small x at ulp=0.5 -> half-integer
  cells -> negative keys -> OOB indirect DMA (wedges the device).
"""

import numpy as np

import concourse.bass as bass
import concourse.tile as tile
from concourse import bacc, mybir
from concourse.bass_utils import run_bass_kernel_spmd
from concourse.tile_rust import add_dep_helper

F32 = mybir.dt.float32
I32 = mybir.dt.int32
AF = mybir.ActivationFunctionType
OP = mybir.AluOpType
AX = mybir.AxisListType

B_TOT = 16
N_CORES = 8
B_SH = B_TOT // N_CORES
NBOX = 64
NP = B_SH * NBOX  # 128 partitions: (image, box)
C = 30
SCALES = [(80, 80), (40, 40), (20, 20)]
NREC = B_SH * 6400  # 12800 rows, one per s0 cell
RW = 36  # per-scale record: obj, reg0..3, cls0..29, pad
LOSE = 1.0e6  # same-cell later-box penalty baked into the utri const
LABB = 2.0e6  # label bias: makes non-equal entries positive in the fused min
MAGIC = 12582912.0  # 1.5*2^23

CLS_W, REG_W, OBJ_W = 1.0, 5.0, 1.0
NPART = 18  # per scale s, cols 6s + [lse, val, sl1, obj, softplus, npos]

# dense obj packing: [128, 132] = s0 cols 0:100 | s1 cols 100:125 | s2 cols 125:132
OBJ_COLS = [(0, 100), (100, 125), (125, 132)]
OBJ_PAD = -1.0e4  # exp -> 0, ln(0+1) -> 0


def emit(tc: tile.TileContext, outs, ins):
    """outs: partials AP [18]; ins: dict name -> AP (per-core shard shapes)."""
    nc = tc.nc
    out_ap = outs

    pools = []

    def mkpool(**kw):
        p = tc.alloc_tile_pool(**kw)
        pools.append(p)
        return p

    pool = mkpool(name="sb", bufs=1)
    psum = mkpool(name="ps", bufs=1, space="PSUM")

    big_c = np.concatenate(
        [np.eye(128, dtype=np.float32), LOSE * np.triu(np.ones((128, 128), np.float32), 1)],
        axis=1,
    )
    big_h = nc.inline_tensor(big_c, name="cbig")
    esel_c = np.zeros((4, 512), np.float32)
    for s in range(4):
        esel_c[s, 128 * s : 128 * (s + 1)] = 1.0
    esel_h = nc.inline_tensor(esel_c, name="cesel")

    # ---- inputs: one packed tile on the critical path, obj on scalar q ----
    pk = pool.tile([128, 48], F32, tag="pk")
    nc.sync.dma_start(out=pk[:], in_=ins["pk"])
    bigt = pool.tile([128, 256], F32, tag="bigt")
    nc.sync.dma_start(out=bigt[:], in_=big_h.ap())
    # [4, 512] row-selector for the broadcast matmuls: row s of block s is 1
    eselt = pool.tile([4, 512], F32, tag="eselt")
    nc.sync.dma_start(out=eselt[:], in_=esel_h.ap())
    objd = pool.tile([128, 132], F32, tag="objd")
    nc.scalar.dma_start(out=objd[:], in_=ins["objdense"])
    ident = bigt[:, 0:128]
    utriL = bigt[:, 128:256]  # utri * LOSE

    # tiny ln bias: keeps ln(ev)=ln(0+eps) finite on loser rows (win=0)
    epst = pool.tile([128, 1], F32, tag="epst")
    nc.vector.memset(epst[:], 1.0e-30)

    # ---- scalar engine: dense-obj exp (Exp table loads at decode) ----
    objE = pool.tile([128, 132], F32, tag="objE")
    i_objE = nc.scalar.activation(out=objE[:], in_=objd[:], func=AF.Exp)

    # ---- box -> cell keys.  Floors stay MAGIC-biased: gr = floor+MAGIC.
    # The s0 key (gather-critical) folds the un-bias into its ops: 6 DVE ops
    # from pk to keyi.  s1/s2 mask keys are derived after the gather issues.
    boxes = pk[:, 0:4]
    kxy = pk[:, 5:11].rearrange("p (c s) -> p c s", c=2)
    gr = pool.tile([NP, 2, 3], F32, tag="gr")
    nc.vector.tensor_tensor(
        out=gr[:], in0=boxes[:, 0:2, None].to_broadcast([NP, 2, 3]), in1=kxy, op=OP.mult
    )
    nc.vector.tensor_scalar(
        out=gr[:], in0=gr[:], scalar1=-0.5, scalar2=MAGIC, op0=OP.add, op1=OP.add
    )
    # kl4 = [keyf0 keyf1 keyf2 | labf]: one transpose feeds all row-matrices
    kl4 = pool.tile([NP, 4], F32, tag="kl4")
    nc.vector.tensor_scalar(
        out=kl4[:, 0:1], in0=gr[:, 1, 0:1], scalar1=-MAGIC, scalar2=80.0,
        op0=OP.add, op1=OP.mult,
    )
    nc.vector.tensor_add(kl4[:, 0:1], kl4[:, 0:1], gr[:, 0, 0:1])
    nc.vector.tensor_scalar(
        out=kl4[:, 0:1], in0=kl4[:, 0:1], scalar1=pk[:, 11:12], scalar2=None, op0=OP.add
    )
    keyi = pool.tile([NP, 1], I32, tag="keyi")
    nc.vector.tensor_copy(out=keyi[:], in_=kl4[:, 0:1])

    # ---- ONE indirect gather: per-box records for all 3 scales.
    # NB the out AP must be 2D [128, 108]: the HW DGE sizes each descriptor
    # by the dest AP's inner dim, not the src row size ----
    og = pool.tile([NP, 3, RW], F32, tag="og")
    nc.gpsimd.indirect_dma_start(
        out=og[:].rearrange("p s r -> p (s r)"),
        out_offset=None,
        in_=ins["rec"],
        in_offset=bass.IndirectOffsetOnAxis(ap=keyi[:], axis=0),
    )

    gf12 = pool.tile([NP, 2, 2], F32, tag="gf12")
    nc.vector.tensor_scalar(
        out=gf12[:], in0=gr[:, :, 1:3], scalar1=-MAGIC, scalar2=None, op0=OP.add
    )
    nc.vector.tensor_tensor(out=kl4[:, 1:3], in0=gf12[:, 1, :], in1=pk[:, 6:8], op=OP.mult)
    nc.vector.tensor_add(kl4[:, 1:3], kl4[:, 1:3], gf12[:, 0, :])
    nc.vector.tensor_add(kl4[:, 1:3], kl4[:, 1:3], pk[:, 12:14])
    nc.vector.tensor_copy(out=kl4[:, 3:4], in_=pk[:, 4:5])

    # ---- key/label row matrices: one PE transpose + 4 selector matmuls ----
    klT_ps = psum.tile([4, 128], F32, tag="klT_ps")
    nc.tensor.transpose(out=klT_ps[:], in_=kl4[:], identity=ident)
    klT = pool.tile([4, 128], F32, tag="klT")
    nc.vector.tensor_copy(out=klT[:], in_=klT_ps[:])
    labps = psum.tile([128, 128], F32, tag="labps")
    nc.tensor.matmul(
        out=labps[:], lhsT=eselt[:, 384:512], rhs=klT[:], start=True, stop=True
    )
    kmat3 = psum.tile([128, 3, 128], F32, tag="kmat3")
    for s in range(3):
        nc.tensor.matmul(
            out=kmat3[:, s, :], lhsT=eselt[:, 128 * s : 128 * (s + 1)], rhs=klT[:],
            start=True, stop=True,
        )

    stack = pool.tile([128, NPART], F32, tag="stack")
    stv = stack[:].rearrange("p (s j) -> p s j", j=6)

    # ---- winners + min same-cell label in ONE reduce: minv[p,s] =
    # min_q( (lab_q + LABB) - LABB*eq - LOSE*(q>p) ).  Labels arrive host-
    # biased by +LABB so equal cells contribute lab - LOSE*(q>p) and
    # non-equal ones stay >= LOSE.  A winner (no later same-cell box) gets
    # its exact min-label in [0,30); a loser goes ~-LOSE.  So win =
    # (minv >= 0), and the one-hot below simply misses for losers (ev=0,
    # made safe by the ln bias). ----
    amat = pool.tile([128, 128], F32, tag="amat")
    nc.vector.tensor_tensor(out=amat[:], in0=labps[:], in1=utriL, op=OP.subtract)
    cnd3 = pool.tile([128, 3, 128], F32, tag="cnd3")
    for s in range(3):
        nc.vector.tensor_scalar(
            out=cnd3[:, s, :], in0=kmat3[:, s, :], scalar1=kl4[:, s : s + 1],
            scalar2=-LABB, op0=OP.is_equal, op1=OP.mult,
        )
    nc.vector.tensor_tensor(
        out=cnd3[:], in0=cnd3[:], in1=amat[:, None, :].to_broadcast([128, 3, 128]),
        op=OP.add,
    )
    minv3 = pool.tile([NP, 3], F32, tag="minv3")
    nc.vector.tensor_reduce(out=minv3[:], in_=cnd3[:], axis=AX.X, op=OP.min)
    nc.vector.tensor_scalar(
        out=stv[:, :, 5], in0=minv3[:], scalar1=0.0, scalar2=None, op0=OP.is_ge
    )
    oh = pool.tile([NP, 3, C], F32, tag="oh")
    nc.vector.tensor_tensor(
        out=oh[:], in0=pk[:, 14:44][:, None, :].to_broadcast([NP, 3, C]),
        in1=minv3[:, :, None].to_broadcast([NP, 3, C]), op=OP.is_equal,
    )

    # ---- cls exp on the gathered records (last Exp op on the queue) ----
    expcls = pool.tile([NP, 3, C], F32, tag="expcls")
    ogv = og[:]
    i_expcls = nc.scalar.activation(out=expcls[:], in_=ogv[:, :, 5:35], func=AF.Exp)
    add_dep_helper(i_expcls.ins, i_objE.ins, reason="scalar q: exps before lns")

    # ---- smooth-L1 over gathered reg records ----
    d3 = pool.tile([NP, 3, 4], F32, tag="d3")
    nc.vector.tensor_tensor(
        out=d3[:], in0=ogv[:, :, 1:5], in1=boxes[:, None, :].to_broadcast([NP, 3, 4]),
        op=OP.subtract,
    )
    dn3 = pool.tile([NP, 3, 4], F32, tag="dn3")
    nc.vector.tensor_scalar(out=dn3[:], in0=d3[:], scalar1=-1.0, scalar2=None, op0=OP.mult)
    nc.vector.tensor_tensor(out=d3[:], in0=d3[:], in1=dn3[:], op=OP.max)
    q3 = pool.tile([NP, 3, 4], F32, tag="q3")
    nc.vector.tensor_scalar_min(q3[:], d3[:], 1.0)
    h3 = pool.tile([NP, 3, 4], F32, tag="h3")
    nc.vector.tensor_scalar(out=h3[:], in0=q3[:], scalar1=-0.5, scalar2=None, op0=OP.mult)
    nc.vector.tensor_add(h3[:], h3[:], d3[:])
    nc.vector.tensor_mul(h3[:], h3[:], q3[:])
    sl3 = pool.tile([NP, 3], F32, tag="sl3")
    nc.vector.tensor_reduce(out=sl3[:], in_=h3[:], axis=AX.X, op=OP.add)
    nc.vector.tensor_scalar(
        out=sl3[:], in0=sl3[:], scalar1=0.25, scalar2=10.0, op0=OP.mult, op1=OP.min
    )

    # ---- logsumexp pieces: se = sum exp(cls), ev = exp(cls[target]) ----
    lsev = pool.tile([NP, 3, 2], F32, tag="lsev")
    nc.vector.tensor_reduce(out=lsev[:, :, 0], in_=expcls[:], axis=AX.X, op=OP.add)
    sel3 = pool.tile([NP, 3, C], F32, tag="sel3")
    nc.vector.tensor_mul(sel3[:], oh[:], expcls[:])
    nc.vector.tensor_reduce(out=lsev[:, :, 1], in_=sel3[:], axis=AX.X, op=OP.add)

    # ---- Ln block (single table load): obj softplus fills the scalar idle
    # window while the DVE reduces lsev; lnv is the last Ln ----
    objL = pool.tile([128, 132], F32, tag="objL")
    prev = i_expcls
    for s, (c0, c1) in enumerate(OBJ_COLS):
        i_l = nc.scalar.activation(
            out=objL[:, c0:c1], in_=objE[:, c0:c1], func=AF.Ln, bias=1.0,
            accum_out=stack[:, 6 * s + 4 : 6 * s + 5],
        )
        add_dep_helper(i_l.ins, prev.ins, reason="scalar q order")
        prev = i_l
    lnv = pool.tile([NP, 3, 2], F32, tag="lnv")
    i_lnv = nc.scalar.activation(out=lnv[:], in_=lsev[:], func=AF.Ln, bias=epst[:])
    add_dep_helper(i_lnv.ins, prev.ins, reason="scalar q: lnv last")

    # ---- stack the win-masked terms ----
    nc.vector.tensor_tensor(
        out=stv[:, :, 0:2], in0=lnv[:], in1=stv[:, :, 5:6].to_broadcast([NP, 3, 2]),
        op=OP.mult,
    )
    nc.vector.tensor_mul(stv[:, :, 2], sl3[:], stv[:, :, 5])
    nc.vector.tensor_mul(stv[:, :, 3], ogv[:, :, 0], stv[:, :, 5])

    # ---- final: ship per-box partial rows; host sums over boxes+cores ----
    nc.sync.dma_start(out=out_ap, in_=stack[:])

    for p in reversed(pools):
        p.release()


# ---------------------------------------------------------------------------
# host side
# ---------------------------------------------------------------------------

_CACHE = {}


def _build():
    if "nc" in _CACHE:
        return _CACHE["nc"]
    nc = bacc.Bacc(
        "TRN2",
        target_bir_lowering=False,
        debug=False,
        enable_asserts=False,
        num_devices=N_CORES,
    )
    ins = {
        "rec": nc.dram_tensor("rec", (NREC, 3 * RW), F32, kind="ExternalInput").ap(),
        "objdense": nc.dram_tensor("objdense", (128, 132), F32, kind="ExternalInput").ap(),
        "pk": nc.dram_tensor("pk", (128, 48), F32, kind="ExternalInput").ap(),
    }
    out = nc.dram_tensor("partials", (128, NPART), F32, kind="ExternalOutput").ap()

    with tile.TileContext(nc) as tc:
        emit(tc, out, ins)
    nc.compile()
    _CACHE["nc"] = nc
    return nc


def _cell_maps():
    """s0-cell row id -> row id in the s1/s2 per-scale record blocks."""
    if "maps" in _CACHE:
        return _CACHE["maps"]
    r = np.arange(NREC)
    u = r // 6400
    y0 = (r % 6400) // 80
    x0 = r % 80
    map1 = u * 1600 + (y0 >> 1) * 40 + (x0 >> 1)
    map2 = u * 400 + (y0 >> 2) * 20 + (x0 >> 2)
    _CACHE["maps"] = (map1, map2)
    return map1, map2


def make_records(cls_sl, reg_sl, obj_sl):
    """[12800, 108]: per s0-cell, records (obj, reg, cls[30], 0) x 3 scales."""
    map1, map2 = _cell_maps()
    blocks = []
    for s, (h, w) in enumerate(SCALES):
        n = B_SH * h * w
        blk = np.zeros((n, RW), np.float32)
        blk[:, 0] = np.asarray(obj_sl[s]).reshape(-1)
        blk[:, 1:5] = (
            np.asarray(reg_sl[s]).reshape(B_SH, 4, h * w).transpose(0, 2, 1).reshape(n, 4)
        )
        blk[:, 5:35] = (
            np.asarray(cls_sl[s]).reshape(B_SH, C, h * w).transpose(0, 2, 1).reshape(n, C)
        )
        blocks.append(blk)
    rec = np.empty((NREC, 3 * RW), np.float32)
    rec[:, 0:36] = blocks[0]
    rec[:, 36:72] = blocks[1][map1]
    rec[:, 72:108] = blocks[2][map2]
    return rec


def make_objdense(obj_sl):
    """Dense obj logits packed [128, 132]; padding -> softplus contributes 0."""
    arr = np.full((128, 132), OBJ_PAD, np.float32)
    for s, (c0, c1) in enumerate(OBJ_COLS):
        v = np.asarray(obj_sl[s]).reshape(-1)
        blk = np.full(128 * (c1 - c0), OBJ_PAD, np.float32)
        blk[: v.size] = v
        arr[:, c0:c1] = blk.reshape(128, c1 - c0)
    return arr


def make_pk(boxes_sl, labels_sl):
    """[128, 48]: boxes | label+LABB | W*3 | H*3 | off0-MAGIC | off1,2 | iota30."""
    pk = np.zeros((128, 48), np.float32)
    pk[:, 0:4] = np.asarray(boxes_sl).reshape(128, 4)
    pk[:, 4] = np.asarray(labels_sl).reshape(128).astype(np.float32) + LABB
    bvec = (np.arange(128) >= NBOX).astype(np.float32)
    for s, (h, w) in enumerate(SCALES):
        pk[:, 5 + s] = w
        pk[:, 8 + s] = h
    pk[:, 11] = bvec * 6400 - MAGIC
    pk[:, 12] = bvec * 1600
    pk[:, 13] = bvec * 400
    pk[:, 14:44] = np.arange(C, dtype=np.float32)[None, :]
    return pk


def combine_partials(parts):
    """parts: [n_cores, 128, 18] -> final [4] losses."""
    tot = np.asarray(parts, np.float64).sum(axis=(0, 1))
    cls_sum = reg_sum = obj_sum = 0.0
    for s, (h, w) in enumerate(SCALES):
        b = 6 * s
        lse, val, sl1, obj, sp, npos = tot[b : b + 6]
        npos = max(npos, 1.0)
        cls_sum += (lse - val) / npos * CLS_W
        reg_sum += sl1 / npos * REG_W
        obj_sum += (sp - obj) / (B_TOT * h * w) * OBJ_W
    cls_sum /= len(SCALES)
    reg_sum /= len(SCALES)
    obj_sum /= len(SCALES)
    total = cls_sum + reg_sum + obj_sum
    return np.array([total, cls_sum, reg_sum, obj_sum], np.float32)


TRACE = False
LAST_RESULT = None


def kernel(**inputs):
    global LAST_RESULT
    nc = _build()
    in_maps = []
    for c in range(N_CORES):
        lo, hi = c * B_SH, (c + 1) * B_SH
        cls_sl = [inputs[f"cls_p{s}"][lo:hi] for s in range(3)]
        reg_sl = [inputs[f"reg_p{s}"][lo:hi] for s in range(3)]
        obj_sl = [inputs[f"obj_p{s}"][lo:hi] for s in range(3)]
        m = {
            "rec": make_records(cls_sl, reg_sl, obj_sl),
            "objdense": make_objdense(obj_sl),
            "pk": make_pk(inputs["boxes"][lo:hi], inputs["labels"][lo:hi]),
        }
        in_maps.append(m)
    res = run_bass_kernel_spmd(
        nc, in_maps, core_ids=list(range(N_CORES)), trace=TRACE
    )
    LAST_RESULT = res
    parts = np.stack([np.asarray(r["partials"]) for r in res.results])
    return combine_partials(parts)


# revision 14
# speedup vs baseline: 1.0157x; 1.0157x over previous
"""DetectionLoss Trainium2 Bass kernel (v3: sparse CE, single gather).

Data-parallel over batch: 2 images per core x 8 cores; host sums 18 partial
sums per core (npos is a global normalizer, so per-core normalization is
impossible anyway - the sharding hint's "per-shard sums + counts").

The cross-entropy term only touches POSITIVE cells (<=128 per scale per
core), so no dense pass over the cls logits is needed.  Host repacks
obj/reg/cls into per-cell records (pure relayout - all arithmetic happens on
device).  Because floor(x*40) == floor(x*80)>>1 exactly in f32 (identical
mantissas), the s1/s2 cells are determined by the s0 cell, so the three
scales' records are concatenated per s0-cell into one [12800, 108] table and
ONE indirect gather fetches obj+reg+cls[30] for all scales.  logsumexp /
smooth-L1 / CE then run on the 128 gathered rows.  Only the objectness BCE
is dense: obj logits arrive packed [128, 132] (pad = -1e4 contributes
softplus 0).

Other latency cuts vs the dense version:
- one packed [128, 48] input DMA (boxes, labels-as-f32, per-scale consts,
  class iota) instead of four small ones: DMA *issue* costs ~0.7us each.
- identity / upper-tri / matmul-selector constants generated on device via
  memset + affine_select instead of DMAed.
- winner masks and min-label reductions batched over scales as [128,3,128];
  the per-scale key/label row-matrices come from one PE transpose of
  [keyf0|keyf1|keyf2|labf] plus 4 selector matmuls.
- scalar queue is forced (false deps) to run Exp ops before all Ln ops:
  the engine has one activation-table slot, each load costs 1.28us.
- floor via (x - 0.5 + 1.5*2^23) - 1.5*2^23: the sum sits in [2^23, 2^24)
  where f32 ulp is 1; plain 2^23 leaves small x at ulp=0.5 -> half-integer
  cells -> negative keys -> OOB indirect DMA (wedges the device).
"""

import numpy as np

import concourse.bass as bass
import concourse.tile as tile
from concourse import bacc, mybir
from concourse.bass_utils import run_bass_kernel_spmd
from concourse.tile_rust import add_dep_helper

F32 = mybir.dt.float32
I32 = mybir.dt.int32
AF = mybir.ActivationFunctionType
OP = mybir.AluOpType
AX = mybir.AxisListType

B_TOT = 16
N_CORES = 8
B_SH = B_TOT // N_CORES
NBOX = 64
NP = B_SH * NBOX  # 128 partitions: (image, box)
C = 30
SCALES = [(80, 80), (40, 40), (20, 20)]
NREC = B_SH * 6400  # 12800 rows, one per s0 cell
RW = 36  # per-scale record: obj, reg0..3, cls0..29, pad
LOSE = 1.0e6  # same-cell later-box penalty baked into the utri const
LABB = 2.0e6  # label bias: makes non-equal entries positive in the fused min
MAGIC = 12582912.0  # 1.5*2^23

CLS_W, REG_W, OBJ_W = 1.0, 5.0, 1.0
NPART = 18  # per scale s, cols 6s + [lse, val, sl1, obj, softplus, npos]

# dense obj packing: [128, 132] = s0 cols 0:100 | s1 cols 100:125 | s2 cols 125:132
OBJ_COLS = [(0, 100), (100, 125), (125, 132)]
OBJ_PAD = -1.0e4  # exp -> 0, ln(0+1) -> 0


def emit(tc: tile.TileContext, outs, ins):
    """outs: partials AP [18]; ins: dict name -> AP (per-core shard shapes)."""
    nc = tc.nc
    out_ap = outs

    pools = []

    def mkpool(**kw):
        p = tc.alloc_tile_pool(**kw)
        pools.append(p)
        return p

    pool = mkpool(name="sb", bufs=1)
    psum = mkpool(name="ps", bufs=1, space="PSUM")

    big_c = np.concatenate(
        [np.eye(128, dtype=np.float32), LOSE * np.triu(np.ones((128, 128), np.float32), 1)],
        axis=1,
    )
    big_h = nc.inline_tensor(big_c, name="cbig")
    esel_c = np.zeros((4, 512), np.float32)
    for s in range(4):
        esel_c[s, 128 * s : 128 * (s + 1)] = 1.0
    esel_h = nc.inline_tensor(esel_c, name="cesel")

    # ---- inputs: one packed tile on the critical path, obj on scalar q ----
    pk = pool.tile([128, 48], F32, tag="pk")
    nc.sync.dma_start(out=pk[:], in_=ins["pk"])
    bigt = pool.tile([128, 256], F32, tag="bigt")
    nc.sync.dma_start(out=bigt[:], in_=big_h.ap())
    # [4, 512] row-selector for the broadcast matmuls: row s of block s is 1
    eselt = pool.tile([4, 512], F32, tag="eselt")
    nc.sync.dma_start(out=eselt[:], in_=esel_h.ap())
    objd = pool.tile([128, 132], F32, tag="objd")
    nc.scalar.dma_start(out=objd[:], in_=ins["objdense"])
    ident = bigt[:, 0:128]
    utriL = bigt[:, 128:256]  # utri * LOSE

    # tiny ln bias: keeps ln(ev)=ln(0+eps) finite on loser rows (win=0)
    epst = pool.tile([128, 1], F32, tag="epst")
    nc.vector.memset(epst[:], 1.0e-30)

    # ---- scalar engine: dense-obj exp (Exp table loads at decode) ----
    objE = pool.tile([128, 132], F32, tag="objE")
    i_objE = nc.scalar.activation(out=objE[:], in_=objd[:], func=AF.Exp)

    # ---- box -> cell keys.  Floors stay MAGIC-biased: gr = floor+MAGIC.
    # The s0 key (gather-critical) folds the un-bias into its ops: 6 DVE ops
    # from pk to keyi.  s1/s2 mask keys are derived after the gather issues.
    boxes = pk[:, 0:4]
    kxy = pk[:, 5:11].rearrange("p (c s) -> p c s", c=2)
    gr = pool.tile([NP, 2, 3], F32, tag="gr")
    nc.vector.tensor_tensor(
        out=gr[:], in0=boxes[:, 0:2, None].to_broadcast([NP, 2, 3]), in1=kxy, op=OP.mult
    )
    nc.vector.tensor_scalar(
        out=gr[:], in0=gr[:], scalar1=-0.5, scalar2=MAGIC, op0=OP.add, op1=OP.add
    )
    # kl4 = [keyf0 keyf1 keyf2 | labf]: one transpose feeds all row-matrices
    kl4 = pool.tile([NP, 4], F32, tag="kl4")
    nc.vector.tensor_scalar(
        out=kl4[:, 0:1], in0=gr[:, 1, 0:1], scalar1=-MAGIC, scalar2=80.0,
        op0=OP.add, op1=OP.mult,
    )
    nc.vector.tensor_add(kl4[:, 0:1], kl4[:, 0:1], gr[:, 0, 0:1])
    nc.vector.tensor_scalar(
        out=kl4[:, 0:1], in0=kl4[:, 0:1], scalar1=pk[:, 11:12], scalar2=None, op0=OP.add
    )
    keyi = pool.tile([NP, 1], I32, tag="keyi")
    i_keyi = nc.vector.tensor_copy(out=keyi[:], in_=kl4[:, 0:1])

    # ---- ONE indirect gather: per-box records for all 3 scales.
    # NB the out AP must be 2D [128, 108]: the HW DGE sizes each descriptor
    # by the dest AP's inner dim, not the src row size ----
    og = pool.tile([NP, 3, RW], F32, tag="og")
    nc.gpsimd.indirect_dma_start(
        out=og[:].rearrange("p s r -> p (s r)"),
        out_offset=None,
        in_=ins["rec"],
        in_offset=bass.IndirectOffsetOnAxis(ap=keyi[:], axis=0),
    )

    gf12 = pool.tile([NP, 2, 2], F32, tag="gf12")
    i_gf = nc.vector.tensor_scalar(
        out=gf12[:], in0=gr[:, :, 1:3], scalar1=-MAGIC, scalar2=None, op0=OP.add
    )
    add_dep_helper(i_gf.ins, i_keyi.ins, reason="DVE q: gather key first")
    nc.vector.tensor_tensor(out=kl4[:, 1:3], in0=gf12[:, 1, :], in1=pk[:, 6:8], op=OP.mult)
    nc.vector.tensor_add(kl4[:, 1:3], kl4[:, 1:3], gf12[:, 0, :])
    nc.vector.tensor_add(kl4[:, 1:3], kl4[:, 1:3], pk[:, 12:14])
    nc.vector.tensor_copy(out=kl4[:, 3:4], in_=pk[:, 4:5])

    # ---- key/label row matrices: one PE transpose + 4 selector matmuls ----
    klT_ps = psum.tile([4, 128], F32, tag="klT_ps")
    nc.tensor.transpose(out=klT_ps[:], in_=kl4[:], identity=ident)
    klT = pool.tile([4, 128], F32, tag="klT")
    nc.vector.tensor_copy(out=klT[:], in_=klT_ps[:])
    labps = psum.tile([128, 128], F32, tag="labps")
    nc.tensor.matmul(
        out=labps[:], lhsT=eselt[:, 384:512], rhs=klT[:], start=True, stop=True
    )
    kmats = []
    for s in range(3):
        km = psum.tile([128, 128], F32, tag=f"kmat{s}")
        nc.tensor.matmul(
            out=km[:], lhsT=eselt[:, 128 * s : 128 * (s + 1)], rhs=klT[:],
            start=True, stop=True,
        )
        kmats.append(km)

    stack = pool.tile([128, NPART], F32, tag="stack")
    stv = stack[:].rearrange("p (s j) -> p s j", j=6)

    # ---- winners + min same-cell label in ONE reduce: minv[p,s] =
    # min_q( (lab_q + LABB) - LABB*eq - LOSE*(q>p) ).  Labels arrive host-
    # biased by +LABB so equal cells contribute lab - LOSE*(q>p) and
    # non-equal ones stay >= LOSE.  A winner (no later same-cell box) gets
    # its exact min-label in [0,30); a loser goes ~-LOSE.  So win =
    # (minv >= 0), and the one-hot below simply misses for losers (ev=0,
    # made safe by the ln bias). ----
    amat = pool.tile([128, 128], F32, tag="amat")
    nc.vector.tensor_tensor(out=amat[:], in0=labps[:], in1=utriL, op=OP.subtract)
    cnd3 = pool.tile([128, 3, 128], F32, tag="cnd3")
    for s in range(3):
        nc.vector.tensor_scalar(
            out=cnd3[:, s, :], in0=kmats[s][:], scalar1=kl4[:, s : s + 1],
            scalar2=-LABB, op0=OP.is_equal, op1=OP.mult,
        )
    nc.vector.tensor_tensor(
        out=cnd3[:], in0=cnd3[:], in1=amat[:, None, :].to_broadcast([128, 3, 128]),
        op=OP.add,
    )
    minv3 = pool.tile([NP, 3], F32, tag="minv3")
    nc.vector.tensor_reduce(out=minv3[:], in_=cnd3[:], axis=AX.X, op=OP.min)
    nc.vector.tensor_scalar(
        out=stv[:, :, 5], in0=minv3[:], scalar1=0.0, scalar2=None, op0=OP.is_ge
    )
    oh = pool.tile([NP, 3, C], F32, tag="oh")
    nc.vector.tensor_tensor(
        out=oh[:], in0=pk[:, 14:44][:, None, :].to_broadcast([NP, 3, C]),
        in1=minv3[:, :, None].to_broadcast([NP, 3, C]), op=OP.is_equal,
    )

    # ---- cls exp on the gathered records (last Exp op on the queue) ----
    expcls = pool.tile([NP, 3, C], F32, tag="expcls")
    ogv = og[:]
    i_expcls = nc.scalar.activation(out=expcls[:], in_=ogv[:, :, 5:35], func=AF.Exp)
    add_dep_helper(i_expcls.ins, i_objE.ins, reason="scalar q: exps before lns")

    # ---- smooth-L1 over gathered reg records ----
    d3 = pool.tile([NP, 3, 4], F32, tag="d3")
    nc.vector.tensor_tensor(
        out=d3[:], in0=ogv[:, :, 1:5], in1=boxes[:, None, :].to_broadcast([NP, 3, 4]),
        op=OP.subtract,
    )
    dn3 = pool.tile([NP, 3, 4], F32, tag="dn3")
    nc.vector.tensor_scalar(out=dn3[:], in0=d3[:], scalar1=-1.0, scalar2=None, op0=OP.mult)
    nc.vector.tensor_tensor(out=d3[:], in0=d3[:], in1=dn3[:], op=OP.max)
    q3 = pool.tile([NP, 3, 4], F32, tag="q3")
    nc.vector.tensor_scalar_min(q3[:], d3[:], 1.0)
    h3 = pool.tile([NP, 3, 4], F32, tag="h3")
    nc.vector.tensor_scalar(out=h3[:], in0=q3[:], scalar1=-0.5, scalar2=None, op0=OP.mult)
    nc.vector.tensor_add(h3[:], h3[:], d3[:])
    nc.vector.tensor_mul(h3[:], h3[:], q3[:])
    sl3 = pool.tile([NP, 3], F32, tag="sl3")
    nc.vector.tensor_reduce(out=sl3[:], in_=h3[:], axis=AX.X, op=OP.add)
    nc.vector.tensor_scalar(
        out=sl3[:], in0=sl3[:], scalar1=0.25, scalar2=10.0, op0=OP.mult, op1=OP.min
    )

    # ---- logsumexp pieces: se = sum exp(cls), ev = exp(cls[target]) ----
    lsev = pool.tile([NP, 3, 2], F32, tag="lsev")
    nc.vector.tensor_reduce(out=lsev[:, :, 0], in_=expcls[:], axis=AX.X, op=OP.add)
    sel3 = pool.tile([NP, 3, C], F32, tag="sel3")
    nc.vector.tensor_mul(sel3[:], oh[:], expcls[:])
    nc.vector.tensor_reduce(out=lsev[:, :, 1], in_=sel3[:], axis=AX.X, op=OP.add)

    # ---- Ln block (single table load): obj softplus fills the scalar idle
    # window while the DVE reduces lsev; lnv is the last Ln ----
    objL = pool.tile([128, 132], F32, tag="objL")
    prev = i_expcls
    for s, (c0, c1) in enumerate(OBJ_COLS):
        i_l = nc.scalar.activation(
            out=objL[:, c0:c1], in_=objE[:, c0:c1], func=AF.Ln, bias=1.0,
            accum_out=stack[:, 6 * s + 4 : 6 * s + 5],
        )
        add_dep_helper(i_l.ins, prev.ins, reason="scalar q order")
        prev = i_l
    lnv = pool.tile([NP, 3, 2], F32, tag="lnv")
    i_lnv = nc.scalar.activation(out=lnv[:], in_=lsev[:], func=AF.Ln, bias=epst[:])
    add_dep_helper(i_lnv.ins, prev.ins, reason="scalar q: lnv last")

    # ---- stack the win-masked terms ----
    nc.vector.tensor_tensor(
        out=stv[:, :, 0:2], in0=lnv[:], in1=stv[:, :, 5:6].to_broadcast([NP, 3, 2]),
        op=OP.mult,
    )
    nc.vector.tensor_mul(stv[:, :, 2], sl3[:], stv[:, :, 5])
    nc.vector.tensor_mul(stv[:, :, 3], ogv[:, :, 0], stv[:, :, 5])

    # ---- final: ship per-box partial rows; host sums over boxes+cores ----
    nc.sync.dma_start(out=out_ap, in_=stack[:])

    for p in reversed(pools):
        p.release()


# ---------------------------------------------------------------------------
# host side
# ---------------------------------------------------------------------------

_CACHE = {}


def _build():
    if "nc" in _CACHE:
        return _CACHE["nc"]
    nc = bacc.Bacc(
        "TRN2",
        target_bir_lowering=False,
        debug=False,
        enable_asserts=False,
        num_devices=N_CORES,
    )
    ins = {
        "rec": nc.dram_tensor("rec", (NREC, 3 * RW), F32, kind="ExternalInput").ap(),
        "objdense": nc.dram_tensor("objdense", (128, 132), F32, kind="ExternalInput").ap(),
        "pk": nc.dram_tensor("pk", (128, 48), F32, kind="ExternalInput").ap(),
    }
    out = nc.dram_tensor("partials", (128, NPART), F32, kind="ExternalOutput").ap()

    with tile.TileContext(nc) as tc:
        emit(tc, out, ins)
    nc.compile()
    _CACHE["nc"] = nc
    return nc


def _cell_maps():
    """s0-cell row id -> row id in the s1/s2 per-scale record blocks."""
    if "maps" in _CACHE:
        return _CACHE["maps"]
    r = np.arange(NREC)
    u = r // 6400
    y0 = (r % 6400) // 80
    x0 = r % 80
    map1 = u * 1600 + (y0 >> 1) * 40 + (x0 >> 1)
    map2 = u * 400 + (y0 >> 2) * 20 + (x0 >> 2)
    _CACHE["maps"] = (map1, map2)
    return map1, map2


def make_records(cls_sl, reg_sl, obj_sl):
    """[12800, 108]: per s0-cell, records (obj, reg, cls[30], 0) x 3 scales."""
    map1, map2 = _cell_maps()
    blocks = []
    for s, (h, w) in enumerate(SCALES):
        n = B_SH * h * w
        blk = np.zeros((n, RW), np.float32)
        blk[:, 0] = np.asarray(obj_sl[s]).reshape(-1)
        blk[:, 1:5] = (
            np.asarray(reg_sl[s]).reshape(B_SH, 4, h * w).transpose(0, 2, 1).reshape(n, 4)
        )
        blk[:, 5:35] = (
            np.asarray(cls_sl[s]).reshape(B_SH, C, h * w).transpose(0, 2, 1).reshape(n, C)
        )
        blocks.append(blk)
    rec = np.empty((NREC, 3 * RW), np.float32)
    rec[:, 0:36] = blocks[0]
    rec[:, 36:72] = blocks[1][map1]
    rec[:, 72:108] = blocks[2][map2]
    return rec


def make_objdense(obj_sl):
    """Dense obj logits packed [128, 132]; padding -> softplus contributes 0."""
    arr = np.full((128, 132), OBJ_PAD, np.float32)
    for s, (c0, c1) in enumerate(OBJ_COLS):
        v = np.asarray(obj_sl[s]).reshape(-1)
        blk = np.full(128 * (c1 - c0), OBJ_PAD, np.float32)
        blk[: v.size] = v
        arr[:, c0:c1] = blk.reshape(128, c1 - c0)
    return arr


def make_pk(boxes_sl, labels_sl):
    """[128, 48]: boxes | label+LABB | W*3 | H*3 | off0-MAGIC | off1,2 | iota30."""
    pk = np.zeros((128, 48), np.float32)
    pk[:, 0:4] = np.asarray(boxes_sl).reshape(128, 4)
    pk[:, 4] = np.asarray(labels_sl).reshape(128).astype(np.float32) + LABB
    bvec = (np.arange(128) >= NBOX).astype(np.float32)
    for s, (h, w) in enumerate(SCALES):
        pk[:, 5 + s] = w
        pk[:, 8 + s] = h
    pk[:, 11] = bvec * 6400 - MAGIC
    pk[:, 12] = bvec * 1600
    pk[:, 13] = bvec * 400
    pk[:, 14:44] = np.arange(C, dtype=np.float32)[None, :]
    return pk


def combine_partials(parts):
    """parts: [n_cores, 128, 18] -> final [4] losses."""
    tot = np.asarray(parts, np.float64).sum(axis=(0, 1))
    cls_sum = reg_sum = obj_sum = 0.0
    for s, (h, w) in enumerate(SCALES):
        b = 6 * s
        lse, val, sl1, obj, sp, npos = tot[b : b + 6]
        npos = max(npos, 1.0)
        cls_sum += (lse - val) / npos * CLS_W
        reg_sum += sl1 / npos * REG_W
        obj_sum += (sp - obj) / (B_TOT * h * w) * OBJ_W
    cls_sum /= len(SCALES)
    reg_sum /= len(SCALES)
    obj_sum /= len(SCALES)
    total = cls_sum + reg_sum + obj_sum
    return np.array([total, cls_sum, reg_sum, obj_sum], np.float32)


TRACE = False
LAST_RESULT = None


def kernel(**inputs):
    global LAST_RESULT
    nc = _build()
    in_maps = []
    for c in range(N_CORES):
        lo, hi = c * B_SH, (c + 1) * B_SH
        cls_sl = [inputs[f"cls_p{s}"][lo:hi] for s in range(3)]
        reg_sl = [inputs[f"reg_p{s}"][lo:hi] for s in range(3)]
        obj_sl = [inputs[f"obj_p{s}"][lo:hi] for s in range(3)]
        m = {
            "rec": make_records(cls_sl, reg_sl, obj_sl),
            "objdense": make_objdense(obj_sl),
            "pk": make_pk(inputs["boxes"][lo:hi], inputs["labels"][lo:hi]),
        }
        in_maps.append(m)
    res = run_bass_kernel_spmd(
        nc, in_maps, core_ids=list(range(N_CORES)), trace=TRACE
    )
    LAST_RESULT = res
    parts = np.stack([np.asarray(r["partials"]) for r in res.results])
    return combine_partials(parts)


# revision 15
# speedup vs baseline: 1.0295x; 1.0136x over previous
"""DetectionLoss Trainium2 Bass kernel (v3: sparse CE, single gather).

Data-parallel over batch: 2 images per core x 8 cores; host sums 18 partial
sums per core (npos is a global normalizer, so per-core normalization is
impossible anyway - the sharding hint's "per-shard sums + counts").

The cross-entropy term only touches POSITIVE cells (<=128 per scale per
core), so no dense pass over the cls logits is needed.  Host repacks
obj/reg/cls into per-cell records (pure relayout - all arithmetic happens on
device).  Because floor(x*40) == floor(x*80)>>1 exactly in f32 (identical
mantissas), the s1/s2 cells are determined by the s0 cell, so the three
scales' records are concatenated per s0-cell into one [12800, 108] table and
ONE indirect gather fetches obj+reg+cls[30] for all scales.  logsumexp /
smooth-L1 / CE then run on the 128 gathered rows.  Only the objectness BCE
is dense: obj logits arrive packed [128, 132] (pad = -1e4 contributes
softplus 0).

Other latency cuts vs the dense version:
- one packed [128, 48] input DMA (boxes, labels-as-f32, per-scale consts,
  class iota) instead of four small ones: DMA *issue* costs ~0.7us each.
- identity / upper-tri / matmul-selector constants generated on device via
  memset + affine_select instead of DMAed.
- winner masks and min-label reductions batched over scales as [128,3,128];
  the per-scale key/label row-matrices come from one PE transpose of
  [keyf0|keyf1|keyf2|labf] plus 4 selector matmuls.
- scalar queue is forced (false deps) to run Exp ops before all Ln ops:
  the engine has one activation-table slot, each load costs 1.28us.
- floor via (x - 0.5 + 1.5*2^23) - 1.5*2^23: the sum sits in [2^23, 2^24)
  where f32 ulp is 1; plain 2^23 leaves small x at ulp=0.5 -> half-integer
  cells -> negative keys -> OOB indirect DMA (wedges the device).
"""

import numpy as np

import concourse.bass as bass
import concourse.tile as tile
from concourse import bacc, mybir
from concourse.bass_utils import run_bass_kernel_spmd
from concourse.tile_rust import add_dep_helper

F32 = mybir.dt.float32
BF16 = mybir.dt.bfloat16
I32 = mybir.dt.int32
AF = mybir.ActivationFunctionType
OP = mybir.AluOpType
AX = mybir.AxisListType

B_TOT = 16
N_CORES = 8
B_SH = B_TOT // N_CORES
NBOX = 64
NP = B_SH * NBOX  # 128 partitions: (image, box)
C = 30
SCALES = [(80, 80), (40, 40), (20, 20)]
NREC = B_SH * 6400  # 12800 rows, one per s0 cell
RW = 36  # per-scale record: obj, reg0..3, cls0..29, pad
LOSE = 1.0  # same-cell later-box penalty baked into the utri const
LABB = 2.0  # label bias: makes non-equal entries positive in the fused min
LSCL = 32.0  # labels are packed as lab/LSCL + LABB: bf16-exact mask math
MAGIC = 12582912.0  # 1.5*2^23

CLS_W, REG_W, OBJ_W = 1.0, 5.0, 1.0
NPART = 18  # per scale s, cols 6s + [lse, val, sl1, obj, softplus, npos]

# dense obj packing: [128, 132] = s0 cols 0:100 | s1 cols 100:125 | s2 cols 125:132
OBJ_COLS = [(0, 100), (100, 125), (125, 132)]
OBJ_PAD = -1.0e4  # exp -> 0, ln(0+1) -> 0


def emit(tc: tile.TileContext, outs, ins):
    """outs: partials AP [18]; ins: dict name -> AP (per-core shard shapes)."""
    nc = tc.nc
    out_ap = outs

    pools = []

    def mkpool(**kw):
        p = tc.alloc_tile_pool(**kw)
        pools.append(p)
        return p

    pool = mkpool(name="sb", bufs=1)
    psum = mkpool(name="ps", bufs=1, space="PSUM")

    big_c = np.concatenate(
        [np.eye(128, dtype=np.float32), np.triu(np.ones((128, 128), np.float32), 1)],
        axis=1,
    )
    big_h = nc.inline_tensor(big_c, name="cbig")
    esel_c = np.zeros((4, 512), np.float32)
    for s in range(4):
        esel_c[s, 128 * s : 128 * (s + 1)] = 1.0
    esel_h = nc.inline_tensor(esel_c, name="cesel")

    # ---- inputs: one packed tile on the critical path, obj on scalar q ----
    pk = pool.tile([128, 48], F32, tag="pk")
    nc.sync.dma_start(out=pk[:], in_=ins["pk"])
    bigt = pool.tile([128, 256], F32, tag="bigt")
    nc.sync.dma_start(out=bigt[:], in_=big_h.ap())
    # [4, 512] row-selector for the broadcast matmuls: row s of block s is 1
    eselt = pool.tile([4, 512], F32, tag="eselt")
    nc.sync.dma_start(out=eselt[:], in_=esel_h.ap())
    objd = pool.tile([128, 132], F32, tag="objd")
    nc.scalar.dma_start(out=objd[:], in_=ins["objdense"])
    ident = bigt[:, 0:128]
    utriL = bigt[:, 128:256]  # utri * LOSE

    # tiny ln bias: keeps ln(ev)=ln(0+eps) finite on loser rows (win=0)
    epst = pool.tile([128, 1], F32, tag="epst")
    nc.vector.memset(epst[:], 1.0e-30)

    # ---- scalar engine: dense-obj exp (Exp table loads at decode) ----
    objE = pool.tile([128, 132], F32, tag="objE")
    i_objE = nc.scalar.activation(out=objE[:], in_=objd[:], func=AF.Exp)

    # ---- box -> cell keys, all 3 scales batched ----
    # floor(x) = (x - 0.5 + 1.5*2^23) - 1.5*2^23: the sum sits in [2^23,2^24)
    # where f32 ulp is 1; plain 2^23 leaves small x at ulp=0.5 -> half-integer
    # cells -> negative keys -> OOB indirect DMA (wedges the device).
    boxes = pk[:, 0:4]
    kxy = pk[:, 5:11].rearrange("p (c s) -> p c s", c=2)
    gr = pool.tile([NP, 2, 3], F32, tag="gr")
    nc.vector.tensor_tensor(
        out=gr[:], in0=boxes[:, 0:2, None].to_broadcast([NP, 2, 3]), in1=kxy, op=OP.mult
    )
    nc.vector.tensor_scalar(
        out=gr[:], in0=gr[:], scalar1=-0.5, scalar2=MAGIC, op0=OP.add, op1=OP.add
    )
    nc.vector.tensor_scalar(out=gr[:], in0=gr[:], scalar1=-MAGIC, scalar2=None, op0=OP.add)
    # kl4 = [keyf0 keyf1 keyf2 | labf]: one transpose feeds all row-matrices
    kl4 = pool.tile([NP, 4], F32, tag="kl4")
    nc.vector.tensor_tensor(out=kl4[:, 0:3], in0=gr[:, 1, :], in1=pk[:, 5:8], op=OP.mult)
    nc.vector.tensor_add(kl4[:, 0:3], kl4[:, 0:3], gr[:, 0, :])
    nc.vector.tensor_add(kl4[:, 0:3], kl4[:, 0:3], pk[:, 11:14])
    keyi = pool.tile([NP, 1], I32, tag="keyi")
    nc.vector.tensor_copy(out=keyi[:], in_=kl4[:, 0:1])

    # ---- ONE indirect gather: per-box records for all 3 scales.
    # NB the out AP must be 2D [128, 108]: the HW DGE sizes each descriptor
    # by the dest AP's inner dim, not the src row size ----
    og = pool.tile([NP, 3, RW], F32, tag="og")
    nc.gpsimd.indirect_dma_start(
        out=og[:].rearrange("p s r -> p (s r)"),
        out_offset=None,
        in_=ins["rec"],
        in_offset=bass.IndirectOffsetOnAxis(ap=keyi[:], axis=0),
    )

    nc.vector.tensor_copy(out=kl4[:, 3:4], in_=pk[:, 4:5])

    # ---- key/label row matrices: one PE transpose + 4 selector matmuls ----
    klT_ps = psum.tile([4, 128], F32, tag="klT_ps")
    nc.tensor.transpose(out=klT_ps[:], in_=kl4[:], identity=ident)
    klT = pool.tile([4, 128], F32, tag="klT")
    nc.vector.tensor_copy(out=klT[:], in_=klT_ps[:])
    labps = psum.tile([128, 128], F32, tag="labps")
    nc.tensor.matmul(
        out=labps[:], lhsT=eselt[:, 384:512], rhs=klT[:], start=True, stop=True
    )
    kmats = []
    for s in range(3):
        km = psum.tile([128, 128], F32, tag=f"kmat{s}")
        nc.tensor.matmul(
            out=km[:], lhsT=eselt[:, 128 * s : 128 * (s + 1)], rhs=klT[:],
            start=True, stop=True,
        )
        kmats.append(km)

    stack = pool.tile([128, NPART], F32, tag="stack")
    stv = stack[:].rearrange("p (s j) -> p s j", j=6)

    # ---- winners + min same-cell label in ONE reduce: minv[p,s] =
    # min_q( (lab_q + LABB) - LABB*eq - LOSE*(q>p) ).  Labels arrive host-
    # biased by +LABB so equal cells contribute lab - LOSE*(q>p) and
    # non-equal ones stay >= LOSE.  A winner (no later same-cell box) gets
    # its exact min-label in [0,30); a loser goes ~-LOSE.  So win =
    # (minv >= 0), and the one-hot below simply misses for losers (ev=0,
    # made safe by the ln bias). ----
    amat = pool.tile([128, 128], BF16, tag="amat")
    nc.vector.tensor_tensor(out=amat[:], in0=labps[:], in1=utriL, op=OP.subtract)
    cnd3 = pool.tile([128, 3, 128], BF16, tag="cnd3")
    for s in range(3):
        nc.vector.tensor_scalar(
            out=cnd3[:, s, :], in0=kmats[s][:], scalar1=kl4[:, s : s + 1],
            scalar2=-LABB, op0=OP.is_equal, op1=OP.mult,
        )
    nc.vector.tensor_tensor(
        out=cnd3[:], in0=cnd3[:], in1=amat[:, None, :].to_broadcast([128, 3, 128]),
        op=OP.add,
    )
    minv3 = pool.tile([NP, 3], F32, tag="minv3")
    nc.vector.tensor_reduce(out=minv3[:], in_=cnd3[:], axis=AX.X, op=OP.min)
    nc.vector.tensor_scalar(
        out=stv[:, :, 5], in0=minv3[:], scalar1=0.0, scalar2=None, op0=OP.is_ge
    )
    oh = pool.tile([NP, 3, C], F32, tag="oh")
    nc.vector.tensor_tensor(
        out=oh[:], in0=pk[:, 14:44][:, None, :].to_broadcast([NP, 3, C]),
        in1=minv3[:, :, None].to_broadcast([NP, 3, C]), op=OP.is_equal,
    )

    # ---- cls exp on the gathered records (last Exp op on the queue) ----
    expcls = pool.tile([NP, 3, C], F32, tag="expcls")
    ogv = og[:]
    i_expcls = nc.scalar.activation(out=expcls[:], in_=ogv[:, :, 5:35], func=AF.Exp)
    add_dep_helper(i_expcls.ins, i_objE.ins, reason="scalar q: exps before lns")

    # ---- smooth-L1 over gathered reg records ----
    d3 = pool.tile([NP, 3, 4], F32, tag="d3")
    nc.vector.tensor_tensor(
        out=d3[:], in0=ogv[:, :, 1:5], in1=boxes[:, None, :].to_broadcast([NP, 3, 4]),
        op=OP.subtract,
    )
    dn3 = pool.tile([NP, 3, 4], F32, tag="dn3")
    nc.vector.tensor_scalar(out=dn3[:], in0=d3[:], scalar1=-1.0, scalar2=None, op0=OP.mult)
    nc.vector.tensor_tensor(out=d3[:], in0=d3[:], in1=dn3[:], op=OP.max)
    q3 = pool.tile([NP, 3, 4], F32, tag="q3")
    nc.vector.tensor_scalar_min(q3[:], d3[:], 1.0)
    h3 = pool.tile([NP, 3, 4], F32, tag="h3")
    nc.vector.tensor_scalar(out=h3[:], in0=q3[:], scalar1=-0.5, scalar2=None, op0=OP.mult)
    nc.vector.tensor_add(h3[:], h3[:], d3[:])
    nc.vector.tensor_mul(h3[:], h3[:], q3[:])
    sl3 = pool.tile([NP, 3], F32, tag="sl3")
    nc.vector.tensor_reduce(out=sl3[:], in_=h3[:], axis=AX.X, op=OP.add)
    nc.vector.tensor_scalar(
        out=sl3[:], in0=sl3[:], scalar1=0.25, scalar2=10.0, op0=OP.mult, op1=OP.min
    )

    # ---- logsumexp pieces: se = sum exp(cls), ev = exp(cls[target]) ----
    lsev = pool.tile([NP, 3, 2], F32, tag="lsev")
    nc.vector.tensor_reduce(out=lsev[:, :, 0], in_=expcls[:], axis=AX.X, op=OP.add)
    sel3 = pool.tile([NP, 3, C], F32, tag="sel3")
    nc.vector.tensor_mul(sel3[:], oh[:], expcls[:])
    nc.vector.tensor_reduce(out=lsev[:, :, 1], in_=sel3[:], axis=AX.X, op=OP.add)

    # ---- Ln block (single table load): obj softplus fills the scalar idle
    # window while the DVE reduces lsev; lnv is the last Ln ----
    objL = pool.tile([128, 132], F32, tag="objL")
    prev = i_expcls
    for s, (c0, c1) in enumerate(OBJ_COLS):
        i_l = nc.scalar.activation(
            out=objL[:, c0:c1], in_=objE[:, c0:c1], func=AF.Ln, bias=1.0,
            accum_out=stack[:, 6 * s + 4 : 6 * s + 5],
        )
        add_dep_helper(i_l.ins, prev.ins, reason="scalar q order")
        prev = i_l
    lnv = pool.tile([NP, 3, 2], F32, tag="lnv")
    i_lnv = nc.scalar.activation(out=lnv[:], in_=lsev[:], func=AF.Ln, bias=epst[:])
    add_dep_helper(i_lnv.ins, prev.ins, reason="scalar q: lnv last")

    # ---- stack the win-masked terms ----
    nc.vector.tensor_tensor(
        out=stv[:, :, 0:2], in0=lnv[:], in1=stv[:, :, 5:6].to_broadcast([NP, 3, 2]),
        op=OP.mult,
    )
    nc.vector.tensor_mul(stv[:, :, 2], sl3[:], stv[:, :, 5])
    nc.vector.tensor_mul(stv[:, :, 3], ogv[:, :, 0], stv[:, :, 5])

    # ---- final: ship per-box partial rows; host sums over boxes+cores ----
    nc.sync.dma_start(out=out_ap, in_=stack[:])

    for p in reversed(pools):
        p.release()


# ---------------------------------------------------------------------------
# host side
# ---------------------------------------------------------------------------

_CACHE = {}


def _build():
    if "nc" in _CACHE:
        return _CACHE["nc"]
    nc = bacc.Bacc(
        "TRN2",
        target_bir_lowering=False,
        debug=False,
        enable_asserts=False,
        num_devices=N_CORES,
    )
    ins = {
        "rec": nc.dram_tensor("rec", (NREC, 3 * RW), F32, kind="ExternalInput").ap(),
        "objdense": nc.dram_tensor("objdense", (128, 132), F32, kind="ExternalInput").ap(),
        "pk": nc.dram_tensor("pk", (128, 48), F32, kind="ExternalInput").ap(),
    }
    out = nc.dram_tensor("partials", (128, NPART), F32, kind="ExternalOutput").ap()

    with tile.TileContext(nc) as tc:
        emit(tc, out, ins)
    nc.compile()
    _CACHE["nc"] = nc
    return nc


def _cell_maps():
    """s0-cell row id -> row id in the s1/s2 per-scale record blocks."""
    if "maps" in _CACHE:
        return _CACHE["maps"]
    r = np.arange(NREC)
    u = r // 6400
    y0 = (r % 6400) // 80
    x0 = r % 80
    map1 = u * 1600 + (y0 >> 1) * 40 + (x0 >> 1)
    map2 = u * 400 + (y0 >> 2) * 20 + (x0 >> 2)
    _CACHE["maps"] = (map1, map2)
    return map1, map2


def make_records(cls_sl, reg_sl, obj_sl):
    """[12800, 108]: per s0-cell, records (obj, reg, cls[30], 0) x 3 scales."""
    map1, map2 = _cell_maps()
    blocks = []
    for s, (h, w) in enumerate(SCALES):
        n = B_SH * h * w
        blk = np.zeros((n, RW), np.float32)
        blk[:, 0] = np.asarray(obj_sl[s]).reshape(-1)
        blk[:, 1:5] = (
            np.asarray(reg_sl[s]).reshape(B_SH, 4, h * w).transpose(0, 2, 1).reshape(n, 4)
        )
        blk[:, 5:35] = (
            np.asarray(cls_sl[s]).reshape(B_SH, C, h * w).transpose(0, 2, 1).reshape(n, C)
        )
        blocks.append(blk)
    rec = np.empty((NREC, 3 * RW), np.float32)
    rec[:, 0:36] = blocks[0]
    rec[:, 36:72] = blocks[1][map1]
    rec[:, 72:108] = blocks[2][map2]
    return rec


def make_objdense(obj_sl):
    """Dense obj logits packed [128, 132]; padding -> softplus contributes 0."""
    arr = np.full((128, 132), OBJ_PAD, np.float32)
    for s, (c0, c1) in enumerate(OBJ_COLS):
        v = np.asarray(obj_sl[s]).reshape(-1)
        blk = np.full(128 * (c1 - c0), OBJ_PAD, np.float32)
        blk[: v.size] = v
        arr[:, c0:c1] = blk.reshape(128, c1 - c0)
    return arr


def make_pk(boxes_sl, labels_sl):
    """[128, 48]: boxes | label+LABB | W*3 | H*3 | off0-MAGIC | off1,2 | iota30."""
    pk = np.zeros((128, 48), np.float32)
    pk[:, 0:4] = np.asarray(boxes_sl).reshape(128, 4)
    pk[:, 4] = np.asarray(labels_sl).reshape(128).astype(np.float32) / LSCL + LABB
    bvec = (np.arange(128) >= NBOX).astype(np.float32)
    for s, (h, w) in enumerate(SCALES):
        pk[:, 5 + s] = w
        pk[:, 8 + s] = h
        pk[:, 11 + s] = bvec * h * w
    pk[:, 14:44] = np.arange(C, dtype=np.float32)[None, :] / LSCL
    return pk


def combine_partials(parts):
    """parts: [n_cores, 128, 18] -> final [4] losses."""
    tot = np.asarray(parts, np.float64).sum(axis=(0, 1))
    cls_sum = reg_sum = obj_sum = 0.0
    for s, (h, w) in enumerate(SCALES):
        b = 6 * s
        lse, val, sl1, obj, sp, npos = tot[b : b + 6]
        npos = max(npos, 1.0)
        cls_sum += (lse - val) / npos * CLS_W
        reg_sum += sl1 / npos * REG_W
        obj_sum += (sp - obj) / (B_TOT * h * w) * OBJ_W
    cls_sum /= len(SCALES)
    reg_sum /= len(SCALES)
    obj_sum /= len(SCALES)
    total = cls_sum + reg_sum + obj_sum
    return np.array([total, cls_sum, reg_sum, obj_sum], np.float32)


TRACE = False
LAST_RESULT = None


def kernel(**inputs):
    global LAST_RESULT
    nc = _build()
    in_maps = []
    for c in range(N_CORES):
        lo, hi = c * B_SH, (c + 1) * B_SH
        cls_sl = [inputs[f"cls_p{s}"][lo:hi] for s in range(3)]
        reg_sl = [inputs[f"reg_p{s}"][lo:hi] for s in range(3)]
        obj_sl = [inputs[f"obj_p{s}"][lo:hi] for s in range(3)]
        m = {
            "rec": make_records(cls_sl, reg_sl, obj_sl),
            "objdense": make_objdense(obj_sl),
            "pk": make_pk(inputs["boxes"][lo:hi], inputs["labels"][lo:hi]),
        }
        in_maps.append(m)
    res = run_bass_kernel_spmd(
        nc, in_maps, core_ids=list(range(N_CORES)), trace=TRACE
    )
    LAST_RESULT = res
    parts = np.stack([np.asarray(r["partials"]) for r in res.results])
    return combine_partials(parts)
